# revision 6
# baseline (speedup 1.0000x reference)
"""BRD4KANModel Trainium2 kernel, v2.

Data-parallel over batch across 8 NeuronCores (512 rows each, weights
replicated). Weights are preprocessed once on the host (static model
weights: scaler and lambda^3 folded into the spline weights, transposed
to contraction-major [k, o], cast to bf16, and packed per (k-tile,
o-block) so each weight DMA is one large contiguous transfer). On-chip
layout is feature-major (h^T: features on partitions, batch on the free
dim). No on-chip weight transposes; the PE runs pure matmul streams.

B-spline bases via the truncated-power identity: with y_m = relu(h-g_m)^3,
bases_ref[c] = lam^3 * (y_c - 4 y_{c+1} + 6 y_{c+2} - 4 y_{c+3} + y_{c+4});
lam^3 is folded into the spline weights on host. The pair sums
p_c = y_c + y_{c+4}, q_c = y_{c+1} + y_{c+3} run on GpSimd (otherwise
idle), the fused (q*-4)+p and (y*6)+t steps on DVE via
scalar_tensor_tensor, batched over all six c in one instruction.

This walrus build accepts only ONE semaphore wait per instruction;
_split_waits() post-processes the BIR JSON, hoisting excess waits onto
NoOps inserted just before each instruction on the same engine.
"""

import json
import os

import numpy as np

import concourse.bass as bass
import concourse.mybir as mybir
import concourse.tile as tile
from concourse.masks import make_identity

F32 = mybir.dt.float32
F16 = mybir.dt.float16
BF16 = mybir.dt.bfloat16
AF = mybir.ActivationFunctionType
OP = mybir.AluOpType

N_CORES = 8
BATCH = 4096
B = BATCH // N_CORES  # 512 per core
D = 2048
WIDTHS = [2048, 2048, 1024]
COEFF = 6
GRID_SIZE = 3
SPLINE_ORDER = 3
H = 2.0 / GRID_SIZE
GRID = [m * H - 1.0 - SPLINE_ORDER * H for m in range(GRID_SIZE + 2 * SPLINE_ORDER + 1)]
LAM3 = 1.0 / (6.0 * H**3)  # lambda^3, folded into spline weights on host

OBLK = 4            # o-tiles per PSUM block
KQ = 4              # k-tiles per quarter (fi=2048 -> 16 k-tiles -> 4 quarters)


def _split_waits(bir_bytes: bytes, keep: int = 1) -> bytes:
    d = json.loads(bir_bytes)
    for f in d["functions"]:
        for bb in f["blocks"]:
            new_insts = []
            for inst in bb["instructions"]:
                si = inst.get("sync_info")
                waits = (si or {}).get("on_wait") or []
                if len(waits) > keep:
                    extra = waits[:-keep]
                    inst["sync_info"]["on_wait"] = waits[-keep:]
                    for ci in range(0, len(extra), keep):
                        new_insts.append({
                            "name": f"{inst['name']}-w{ci}",
                            "opcode": "NoOp",
                            "engine": inst["engine"],
                            "ins": [],
                            "outs": [],
                            "debug": inst.get("debug"),
                            "sync_info": {"on_update": [],
                                          "on_wait": extra[ci:ci + keep]},
                        })
                new_insts.append(inst)
            bb["instructions"] = new_insts
    return json.dumps(d).encode()


def _patch_json(nc):
    orig = nc.to_json_bytes

    def patched():
        return _split_waits(orig())

    nc.to_json_bytes = patched
    return nc


def build():
    nc = bass.Bass()
    dims = [D] + WIDTHS
    x = nc.dram_tensor("x", [B, D], F32, kind="ExternalInput")
    wm = nc.dram_tensor("wm", [16, 8, 128, 512], BF16, kind="ExternalInput")
    mb = nc.dram_tensor("mb", [2 * D], F32, kind="ExternalInput")
    wk = []
    for l in range(3):
        nblk = dims[l + 1] // (OBLK * 128)
        wk.append(nc.dram_tensor(f"wk{l}", [16, nblk, 7, 128, OBLK * 128],
                                 BF16, kind="ExternalInput"))
    hw = nc.dram_tensor("hw", [WIDTHS[-1], 2], BF16, kind="ExternalInput")
    hb = nc.dram_tensor("hb", [2, 1], F32, kind="ExternalInput")
    out = nc.dram_tensor("out", [2, B], F32, kind="ExternalOutput")

    with tile.TileContext(nc) as tc:
        with tc.tile_pool(name="consts", bufs=1) as consts, \
             tc.tile_pool(name="hring", bufs=2) as hring, \
             tc.tile_pool(name="h2p", bufs=1) as h2p, \
             tc.tile_pool(name="basesp", bufs=4) as basesp, \
             tc.tile_pool(name="yp", bufs=2) as yp, \
             tc.tile_pool(name="pp", bufs=1) as pp, \
             tc.tile_pool(name="qp", bufs=1) as qp, \
             tc.tile_pool(name="rp", bufs=2) as rp, \
             tc.tile_pool(name="xbp", bufs=16) as xbp, \
             tc.tile_pool(name="silup", bufs=2) as silup, \
             tc.tile_pool(name="wkp", bufs=2) as wkp, \
             tc.tile_pool(name="wmp", bufs=4) as wmp, \
             tc.tile_pool(name="xfp", bufs=1) as xfp, \
             tc.tile_pool(name="psA", bufs=6, space="PSUM") as psA, \
             tc.tile_pool(name="psT", bufs=2, space="PSUM") as psT:

            # ---- constants ----
            ident = consts.tile([128, 128], BF16, tag="ident")
            make_identity(nc, ident)
            mb_sb = consts.tile([128, 32], F32, tag="mb")
            nc.sync.dma_start(mb_sb, mb[:].rearrange("(t p) -> p t", p=128))
            hw_sb = consts.tile([128, 8, 2], BF16, tag="hw")
            nc.sync.dma_start(hw_sb, hw[:].rearrange("(t p) c -> p t c", p=128))
            hb_sb = consts.tile([2, 1], F32, tag="hb")
            nc.sync.dma_start(hb_sb, hb[:])
            grid_sb = consts.tile([128, 10], F32, tag="grid")
            for m in range(10):
                nc.vector.memset(grid_sb[:, m:m + 1], float(-GRID[m]))

            # ---- x: load, cast bf16, PE-transpose to feature-major ----
            xb = [xbp.tile([128, B], BF16, tag="xb", name=f"xb{i}")
                  for i in range(16)]
            for bt in range(4):
                xf = xfp.tile([128, D], BF16, tag="xf")
                nc.gpsimd.dma_start(xf, x[bt * 128:(bt + 1) * 128, :])
                for g in range(4):
                    pt = psT.tile([128, 512], BF16, tag="pt")
                    for qq in range(4):
                        i = 4 * g + qq
                        nc.tensor.transpose(pt[:, qq * 128:(qq + 1) * 128],
                                            xf[:, i * 128:(i + 1) * 128], ident)
                    for qq in range(4):
                        i = 4 * g + qq
                        nc.scalar.copy(xb[i][:, bt * 128:(bt + 1) * 128],
                                       pt[:, qq * 128:(qq + 1) * 128])

            # ---- multiplicative layer ----
            # h0 = sigmoid(x@Wg + bg) * (x@Wv + bv), feature-major fp16
            sig_t = hring.tile([128, 16, B], F16, tag="h", name="sig")
            h_in = hring.tile([128, 16, B], F16, tag="h", name="h0")

            def mult_block(blk, is_gate, j):
                accs = [psA.tile([128, B], F32, tag="acc", name=f"m{blk}_{o}")
                        for o in range(OBLK)]
                for kt in range(16):
                    ws = wmp.tile([128, 512], BF16, tag="wm")
                    nc.sync.dma_start(ws, wm[kt, blk])
                    for o in range(OBLK):
                        nc.tensor.matmul(accs[o], ws[:, o * 128:(o + 1) * 128],
                                         xb[kt], start=(kt == 0),
                                         stop=(kt == 15))
                for o in range(OBLK):
                    ot = 4 * j + o
                    if is_gate:
                        nc.scalar.activation(sig_t[:, ot, :], accs[o],
                                             AF.Sigmoid,
                                             bias=mb_sb[:, ot:ot + 1])
                    else:
                        nc.vector.scalar_tensor_tensor(
                            h_in[:, ot, :], accs[o], mb_sb[:, 16 + ot:17 + ot],
                            sig_t[:, ot, :], OP.add, OP.mult)

            for j in range(4):
                mult_block(j, True, j)        # gate o-tiles 4j..4j+3
                mult_block(4 + j, False, j)   # val  o-tiles 4j..4j+3

            # ---- KAN layers ----
            def z_quarter(l, q, h_t):
                """bases + silu for k-tiles [4q, 4q+4) of layer l.
                Returns (silu tile [128,4,B], [bases tile per pair])."""
                st = silup.tile([128, KQ, B], BF16, tag="silu",
                                name=f"silu{l}_{q}")
                nc.scalar.activation(st, h_t[:, 4 * q:4 * q + 4, :], AF.Silu)
                pair_tiles = []
                for tpair in (2 * q, 2 * q + 1):
                    bt6 = basesp.tile([128, COEFF, 2, B], BF16, tag="bases",
                                      name=f"bas{l}_{tpair}")
                    pair_tiles.append(bt6)
                    for bh in range(2):
                        sl = slice(bh * 256, (bh + 1) * 256)
                        hsl = h_t[:, 2 * tpair:2 * tpair + 2, sl]
                        y = yp.tile([128, 10, 2, 256], F32, tag="y")
                        yf = y.rearrange("p m i b -> p (m i b)")
                        for m in range(10):
                            r = rp.tile([128, 2, 256], F32, tag="r")
                            rf = r.rearrange("p i b -> p (i b)")
                            nc.scalar.activation(r, hsl, AF.Relu,
                                                 bias=grid_sb[:, m:m + 1])
                            nc.scalar.activation(y[:, m], r, AF.Square)
                            nc.vector.tensor_tensor(
                                yf[:, m * 512:(m + 1) * 512],
                                yf[:, m * 512:(m + 1) * 512], rf, OP.mult)
                        p = pp.tile([128, COEFF, 2, 256], F32, tag="p")
                        pf = p.rearrange("p c i b -> p (c i b)")
                        nc.gpsimd.tensor_tensor(pf, yf[:, 0:3072],
                                                yf[:, 2048:5120], OP.add)
                        qq = qp.tile([128, COEFF, 2, 256], F32, tag="q")
                        qf = qq.rearrange("p c i b -> p (c i b)")
                        nc.gpsimd.tensor_tensor(qf, yf[:, 512:3584],
                                                yf[:, 1536:4608], OP.add)
                        nc.vector.scalar_tensor_tensor(pf, qf, -4.0, pf,
                                                       OP.mult, OP.add)
                        nc.vector.scalar_tensor_tensor(
                            bt6[:, :, :, sl], yf[:, 1024:4096].rearrange(
                                "p (c i b) -> p c i b", c=6, i=2), 6.0, p,
                            OP.mult, OP.add)
                return st, pair_tiles

            def sweep_quarter(l, q, hacc, h2_t, silu_t, pairs):
                """matmuls for k-quarter q over all o of layer l."""
                fo = dims[l + 1]
                nblk = fo // (OBLK * 128)
                last_q = (q == 3)
                for blk in range(nblk):
                    accs = [psA.tile([128, B], F32, tag="acc",
                                     name=f"a{l}_{q}_{blk}_{o}")
                            for o in range(OBLK)]
                    for ki in range(KQ):
                        kt = 4 * q + ki
                        wt = wkp.tile([128, 7, OBLK * 128], BF16, tag="wk")
                        nc.sync.dma_start(
                            wt, wk[l][kt, blk].rearrange("s p f -> p s f"))
                        for o in range(OBLK):
                            osl = slice(o * 128, (o + 1) * 128)
                            for s in range(7):
                                rhs = (silu_t[:, ki, :] if s == 0
                                       else pairs[ki // 2][:, s - 1, ki % 2, :])
                                nc.tensor.matmul(
                                    accs[o], wt[:, s, osl], rhs,
                                    start=(ki == 0 and s == 0),
                                    stop=(ki == KQ - 1 and s == 6))
                    for o in range(OBLK):
                        ot = OBLK * blk + o
                        if q == 0:
                            nc.scalar.copy(hacc[:, ot, :], accs[o])
                        elif last_q and l == 2:
                            nc.vector.tensor_tensor(h2_t[:, ot, :], accs[o],
                                                    hacc[:, ot, :], OP.add)
                        else:
                            nc.vector.tensor_tensor(hacc[:, ot, :], accs[o],
                                                    hacc[:, ot, :], OP.add)

            h2_t = h2p.tile([128, 8, B], BF16, tag="h2")
            for l in range(3):
                hacc = hring.tile([128, 16, B], F16, tag="h", name=f"hacc{l}")
                s0, p0 = z_quarter(l, 0, h_in)
                s1, p1 = z_quarter(l, 1, h_in)
                zq = [(s0, p0), (s1, p1)]
                for q in range(4):
                    if q + 2 < 4:
                        zq.append(z_quarter(l, q + 2, h_in))
                    st, pairs = zq[q]
                    sweep_quarter(l, q, hacc, h2_t, st, pairs)
                h_in = hacc

            # ---- heads ----
            acc2 = psA.tile([128, B], F32, tag="acc", name="headacc")
            for kt in range(8):
                nc.tensor.matmul(acc2[0:2, :], hw_sb[:, kt, :],
                                 h2_t[:, kt, :], start=(kt == 0),
                                 stop=(kt == 7))
            res = consts.tile([2, B], F32, tag="res")
            nc.vector.tensor_scalar(res, acc2[0:2, :], hb_sb[:, 0:1], None,
                                    OP.add)
            nc.sync.dma_start(out[:], res)

    return _patch_json(nc)


_NC = None
_PACKED = None


def _pack_weights(inputs):
    import ml_dtypes
    bf16 = ml_dtypes.bfloat16
    dims = [D] + WIDTHS
    packed = {}
    # mult: wT[i, o] -> [16, 8, 128, 512]
    wT = np.ascontiguousarray(np.asarray(inputs["mult_w"], np.float32).T)
    packed["wm"] = np.ascontiguousarray(
        wT.reshape(16, 128, 8, 4, 128).transpose(0, 2, 1, 3, 4)
        .reshape(16, 8, 128, 512)).astype(bf16)
    packed["mb"] = np.ascontiguousarray(np.asarray(inputs["mult_b"], np.float32))
    for l in range(3):
        fi, fo = dims[l], dims[l + 1]
        nblk = fo // (OBLK * 128)
        bw = np.asarray(inputs[f"base_w{l}"], np.float32)
        sw = np.asarray(inputs[f"spline_w{l}"], np.float32)
        sc = np.asarray(inputs[f"scaler{l}"], np.float32)
        S = np.empty((fi, 7, fo), np.float32)
        S[:, 0, :] = bw.T
        S[:, 1:, :] = (sw * (sc[:, :, None] * LAM3)).transpose(1, 2, 0)
        packed[f"wk{l}"] = np.ascontiguousarray(
            S.reshape(16, 128, 7, nblk, OBLK * 128).transpose(0, 3, 2, 1, 4)
        ).astype(bf16)
    packed["hw"] = np.ascontiguousarray(np.stack(
        [np.asarray(inputs["reg_w"], np.float32)[0],
         np.asarray(inputs["aux_w"], np.float32)[0]], axis=1)).astype(bf16)
    packed["hb"] = np.array(
        [[float(np.asarray(inputs["reg_b"]).reshape(-1)[0])],
         [float(np.asarray(inputs["aux_b"]).reshape(-1)[0])]], np.float32)
    return packed


def kernel(**inputs):
    global _NC, _PACKED
    from concourse.bass_utils import run_bass_kernel_spmd

    if _NC is None:
        _NC = build()
    if _PACKED is None:
        _PACKED = _pack_weights(inputs)
    x_full = np.ascontiguousarray(np.asarray(inputs["x"], np.float32))
    per_core = []
    for c in range(N_CORES):
        m = dict(_PACKED)
        m["x"] = np.ascontiguousarray(x_full[c * B:(c + 1) * B])
        per_core.append(m)
    res = run_bass_kernel_spmd(_NC, per_core, core_ids=list(range(N_CORES)))
    reg = np.concatenate([res.results[c]["out"][0] for c in range(N_CORES)])
    aux = np.concatenate([res.results[c]["out"][1] for c in range(N_CORES)])
    kernel.last_results = res
    return reg, aux


# revision 7
# speedup vs baseline: 1.0174x; 1.0174x over previous
"""BRD4KANModel Trainium2 kernel, v2.

Data-parallel over batch across 8 NeuronCores (512 rows each, weights
replicated). Weights are preprocessed once on the host (static model
weights: scaler and lambda^3 folded into the spline weights, transposed
to contraction-major [k, o], cast to bf16, and packed per (k-tile,
o-block) so each weight DMA is one large contiguous transfer). On-chip
layout is feature-major (h^T: features on partitions, batch on the free
dim). No on-chip weight transposes; the PE runs pure matmul streams.

B-spline bases via the truncated-power identity: with y_m = relu(h-g_m)^3,
bases_ref[c] = lam^3 * (y_c - 4 y_{c+1} + 6 y_{c+2} - 4 y_{c+3} + y_{c+4});
lam^3 is folded into the spline weights on host. The pair sums
p_c = y_c + y_{c+4}, q_c = y_{c+1} + y_{c+3} run on GpSimd (otherwise
idle), the fused (q*-4)+p and (y*6)+t steps on DVE via
scalar_tensor_tensor, batched over all six c in one instruction.

This walrus build accepts only ONE semaphore wait per instruction;
_split_waits() post-processes the BIR JSON, hoisting excess waits onto
NoOps inserted just before each instruction on the same engine.
"""

import json
import os

import numpy as np

import concourse.bass as bass
import concourse.mybir as mybir
import concourse.tile as tile
from concourse.masks import make_identity

F32 = mybir.dt.float32
F16 = mybir.dt.float16
BF16 = mybir.dt.bfloat16
AF = mybir.ActivationFunctionType
OP = mybir.AluOpType

N_CORES = 8
BATCH = 4096
B = BATCH // N_CORES  # 512 per core
D = 2048
WIDTHS = [2048, 2048, 1024]
COEFF = 6
GRID_SIZE = 3
SPLINE_ORDER = 3
H = 2.0 / GRID_SIZE
GRID = [m * H - 1.0 - SPLINE_ORDER * H for m in range(GRID_SIZE + 2 * SPLINE_ORDER + 1)]
LAM3 = 1.0 / (6.0 * H**3)  # lambda^3, folded into spline weights on host

OBLK = 4            # o-tiles per PSUM block
KQ = 4              # k-tiles per quarter (fi=2048 -> 16 k-tiles -> 4 quarters)


def _split_waits(bir_bytes: bytes, keep: int = 1) -> bytes:
    d = json.loads(bir_bytes)
    for f in d["functions"]:
        for bb in f["blocks"]:
            new_insts = []
            for inst in bb["instructions"]:
                si = inst.get("sync_info")
                waits = (si or {}).get("on_wait") or []
                if len(waits) > keep:
                    extra = waits[:-keep]
                    inst["sync_info"]["on_wait"] = waits[-keep:]
                    for ci in range(0, len(extra), keep):
                        new_insts.append({
                            "name": f"{inst['name']}-w{ci}",
                            "opcode": "NoOp",
                            "engine": inst["engine"],
                            "ins": [],
                            "outs": [],
                            "debug": inst.get("debug"),
                            "sync_info": {"on_update": [],
                                          "on_wait": extra[ci:ci + keep]},
                        })
                new_insts.append(inst)
            bb["instructions"] = new_insts
    return json.dumps(d).encode()


def _patch_json(nc):
    orig = nc.to_json_bytes

    def patched():
        return _split_waits(orig())

    nc.to_json_bytes = patched
    return nc


def build():
    nc = bass.Bass()
    dims = [D] + WIDTHS
    x = nc.dram_tensor("x", [B, D], F32, kind="ExternalInput")
    wm = nc.dram_tensor("wm", [16, 8, 128, 512], BF16, kind="ExternalInput")
    mb = nc.dram_tensor("mb", [2 * D], F32, kind="ExternalInput")
    wk = []
    for l in range(3):
        nblk = dims[l + 1] // (OBLK * 128)
        wk.append(nc.dram_tensor(f"wk{l}", [16, nblk, 7, 128, OBLK * 128],
                                 BF16, kind="ExternalInput"))
    hw = nc.dram_tensor("hw", [WIDTHS[-1], 2], BF16, kind="ExternalInput")
    hb = nc.dram_tensor("hb", [2, 1], F32, kind="ExternalInput")
    out = nc.dram_tensor("out", [2, B], F32, kind="ExternalOutput")

    with tile.TileContext(nc) as tc:
        with tc.tile_pool(name="consts", bufs=1) as consts, \
             tc.tile_pool(name="hring", bufs=2) as hring, \
             tc.tile_pool(name="h2p", bufs=1) as h2p, \
             tc.tile_pool(name="basesp", bufs=4) as basesp, \
             tc.tile_pool(name="yp", bufs=2) as yp, \
             tc.tile_pool(name="pp", bufs=1) as pp, \
             tc.tile_pool(name="qp", bufs=1) as qp, \
             tc.tile_pool(name="rp", bufs=2) as rp, \
             tc.tile_pool(name="xbp", bufs=16) as xbp, \
             tc.tile_pool(name="silup", bufs=2) as silup, \
             tc.tile_pool(name="wkp", bufs=2) as wkp, \
             tc.tile_pool(name="wmp", bufs=4) as wmp, \
             tc.tile_pool(name="xfp", bufs=1) as xfp, \
             tc.tile_pool(name="psA", bufs=6, space="PSUM") as psA, \
             tc.tile_pool(name="psT", bufs=2, space="PSUM") as psT:

            # ---- constants ----
            ident = consts.tile([128, 128], BF16, tag="ident")
            make_identity(nc, ident)
            mb_sb = consts.tile([128, 32], F32, tag="mb")
            nc.sync.dma_start(mb_sb, mb[:].rearrange("(t p) -> p t", p=128))
            hw_sb = consts.tile([128, 8, 2], BF16, tag="hw")
            nc.sync.dma_start(hw_sb, hw[:].rearrange("(t p) c -> p t c", p=128))
            hb_sb = consts.tile([2, 1], F32, tag="hb")
            nc.sync.dma_start(hb_sb, hb[:])
            grid_sb = consts.tile([128, 10], F32, tag="grid")
            for m in range(10):
                nc.vector.memset(grid_sb[:, m:m + 1], float(-GRID[m]))

            # ---- x: load, cast bf16, PE-transpose to feature-major ----
            xb = [xbp.tile([128, B], BF16, tag="xb", name=f"xb{i}")
                  for i in range(16)]
            for bt in range(4):
                xf = xfp.tile([128, D], BF16, tag="xf")
                nc.gpsimd.dma_start(xf, x[bt * 128:(bt + 1) * 128, :])
                for g in range(4):
                    pt = psT.tile([128, 512], BF16, tag="pt")
                    for qq in range(4):
                        i = 4 * g + qq
                        nc.tensor.transpose(pt[:, qq * 128:(qq + 1) * 128],
                                            xf[:, i * 128:(i + 1) * 128], ident)
                    for qq in range(4):
                        i = 4 * g + qq
                        nc.scalar.copy(xb[i][:, bt * 128:(bt + 1) * 128],
                                       pt[:, qq * 128:(qq + 1) * 128])

            # ---- multiplicative layer ----
            # h0 = sigmoid(x@Wg + bg) * (x@Wv + bv), feature-major fp16
            sig_t = hring.tile([128, 16, B], F16, tag="h", name="sig")
            h_in = hring.tile([128, 16, B], F16, tag="h", name="h0")

            def mult_block(blk, is_gate, j):
                accs = [psA.tile([128, B], F32, tag="acc", name=f"m{blk}_{o}")
                        for o in range(OBLK)]
                for kt in range(16):
                    ws = wmp.tile([128, 512], BF16, tag="wm")
                    nc.sync.dma_start(ws, wm[kt, blk])
                    for o in range(OBLK):
                        nc.tensor.matmul(accs[o], ws[:, o * 128:(o + 1) * 128],
                                         xb[kt], start=(kt == 0),
                                         stop=(kt == 15))
                for o in range(OBLK):
                    ot = 4 * j + o
                    if is_gate:
                        nc.scalar.activation(sig_t[:, ot, :], accs[o],
                                             AF.Sigmoid,
                                             bias=mb_sb[:, ot:ot + 1])
                    else:
                        nc.vector.scalar_tensor_tensor(
                            h_in[:, ot, :], accs[o], mb_sb[:, 16 + ot:17 + ot],
                            sig_t[:, ot, :], OP.add, OP.mult)

            for j in range(4):
                mult_block(j, True, j)        # gate o-tiles 4j..4j+3
                mult_block(4 + j, False, j)   # val  o-tiles 4j..4j+3

            # ---- KAN layers ----
            def z_quarter(l, q, h_t):
                """bases + silu for k-tiles [4q, 4q+4) of layer l.
                Returns (silu tile [128,4,B], [bases tile per pair])."""
                st = silup.tile([128, KQ, B], BF16, tag="silu",
                                name=f"silu{l}_{q}")
                nc.scalar.activation(st, h_t[:, 4 * q:4 * q + 4, :], AF.Silu)
                pair_tiles = []
                for tpair in (2 * q, 2 * q + 1):
                    bt6 = basesp.tile([128, COEFF, 2, B], BF16, tag="bases",
                                      name=f"bas{l}_{tpair}")
                    pair_tiles.append(bt6)
                    for bh in range(2):
                        sl = slice(bh * 256, (bh + 1) * 256)
                        hsl = h_t[:, 2 * tpair:2 * tpair + 2, sl]
                        y = yp.tile([128, 10, 2, 256], F32, tag="y")
                        yf = y.rearrange("p m i b -> p (m i b)")
                        for m in range(10):
                            r = rp.tile([128, 2, 256], F32, tag="r")
                            nc.scalar.activation(r, hsl, AF.Relu,
                                                 bias=grid_sb[:, m:m + 1])
                            nc.scalar.activation(y[:, m], r, AF.Square)
                            # y_m = (h - g_m) * relu(h - g_m)^2 == relu(.)^3
                            nc.vector.scalar_tensor_tensor(
                                y[:, m], hsl, grid_sb[:, m:m + 1], y[:, m],
                                OP.add, OP.mult)
                        p = pp.tile([128, COEFF, 2, 256], F32, tag="p")
                        pf = p.rearrange("p c i b -> p (c i b)")
                        nc.gpsimd.tensor_tensor(pf, yf[:, 0:3072],
                                                yf[:, 2048:5120], OP.add)
                        qq = qp.tile([128, COEFF, 2, 256], F32, tag="q")
                        qf = qq.rearrange("p c i b -> p (c i b)")
                        nc.gpsimd.tensor_tensor(qf, yf[:, 512:3584],
                                                yf[:, 1536:4608], OP.add)
                        nc.vector.scalar_tensor_tensor(pf, qf, -4.0, pf,
                                                       OP.mult, OP.add)
                        nc.vector.scalar_tensor_tensor(
                            bt6[:, :, :, sl], yf[:, 1024:4096].rearrange(
                                "p (c i b) -> p c i b", c=6, i=2), 6.0, p,
                            OP.mult, OP.add)
                return st, pair_tiles

            def sweep_quarter(l, q, hacc, h2_t, silu_t, pairs):
                """matmuls for k-quarter q over all o of layer l."""
                fo = dims[l + 1]
                nblk = fo // (OBLK * 128)
                last_q = (q == 3)
                for blk in range(nblk):
                    accs = [psA.tile([128, B], F32, tag="acc",
                                     name=f"a{l}_{q}_{blk}_{o}")
                            for o in range(OBLK)]
                    for ki in range(KQ):
                        kt = 4 * q + ki
                        wt = wkp.tile([128, 7, OBLK * 128], BF16, tag="wk")
                        nc.sync.dma_start(
                            wt, wk[l][kt, blk].rearrange("s p f -> p s f"))
                        for o in range(OBLK):
                            osl = slice(o * 128, (o + 1) * 128)
                            for s in range(7):
                                rhs = (silu_t[:, ki, :] if s == 0
                                       else pairs[ki // 2][:, s - 1, ki % 2, :])
                                nc.tensor.matmul(
                                    accs[o], wt[:, s, osl], rhs,
                                    start=(ki == 0 and s == 0),
                                    stop=(ki == KQ - 1 and s == 6))
                    for o in range(OBLK):
                        ot = OBLK * blk + o
                        if q == 0:
                            nc.scalar.copy(hacc[:, ot, :], accs[o])
                        elif last_q and l == 2:
                            nc.vector.tensor_tensor(h2_t[:, ot, :], accs[o],
                                                    hacc[:, ot, :], OP.add)
                        else:
                            nc.vector.tensor_tensor(hacc[:, ot, :], accs[o],
                                                    hacc[:, ot, :], OP.add)

            h2_t = h2p.tile([128, 8, B], BF16, tag="h2")
            for l in range(3):
                hacc = hring.tile([128, 16, B], F16, tag="h", name=f"hacc{l}")
                s0, p0 = z_quarter(l, 0, h_in)
                s1, p1 = z_quarter(l, 1, h_in)
                zq = [(s0, p0), (s1, p1)]
                for q in range(4):
                    if q + 2 < 4:
                        zq.append(z_quarter(l, q + 2, h_in))
                    st, pairs = zq[q]
                    sweep_quarter(l, q, hacc, h2_t, st, pairs)
                h_in = hacc

            # ---- heads ----
            acc2 = psA.tile([128, B], F32, tag="acc", name="headacc")
            for kt in range(8):
                nc.tensor.matmul(acc2[0:2, :], hw_sb[:, kt, :],
                                 h2_t[:, kt, :], start=(kt == 0),
                                 stop=(kt == 7))
            res = consts.tile([2, B], F32, tag="res")
            nc.vector.tensor_scalar(res, acc2[0:2, :], hb_sb[:, 0:1], None,
                                    OP.add)
            nc.sync.dma_start(out[:], res)

    return _patch_json(nc)


_NC = None
_PACKED = None


def _pack_weights(inputs):
    import ml_dtypes
    bf16 = ml_dtypes.bfloat16
    dims = [D] + WIDTHS
    packed = {}
    # mult: wT[i, o] -> [16, 8, 128, 512]
    wT = np.ascontiguousarray(np.asarray(inputs["mult_w"], np.float32).T)
    packed["wm"] = np.ascontiguousarray(
        wT.reshape(16, 128, 8, 4, 128).transpose(0, 2, 1, 3, 4)
        .reshape(16, 8, 128, 512)).astype(bf16)
    packed["mb"] = np.ascontiguousarray(np.asarray(inputs["mult_b"], np.float32))
    for l in range(3):
        fi, fo = dims[l], dims[l + 1]
        nblk = fo // (OBLK * 128)
        bw = np.asarray(inputs[f"base_w{l}"], np.float32)
        sw = np.asarray(inputs[f"spline_w{l}"], np.float32)
        sc = np.asarray(inputs[f"scaler{l}"], np.float32)
        S = np.empty((fi, 7, fo), np.float32)
        S[:, 0, :] = bw.T
        S[:, 1:, :] = (sw * (sc[:, :, None] * LAM3)).transpose(1, 2, 0)
        packed[f"wk{l}"] = np.ascontiguousarray(
            S.reshape(16, 128, 7, nblk, OBLK * 128).transpose(0, 3, 2, 1, 4)
        ).astype(bf16)
    packed["hw"] = np.ascontiguousarray(np.stack(
        [np.asarray(inputs["reg_w"], np.float32)[0],
         np.asarray(inputs["aux_w"], np.float32)[0]], axis=1)).astype(bf16)
    packed["hb"] = np.array(
        [[float(np.asarray(inputs["reg_b"]).reshape(-1)[0])],
         [float(np.asarray(inputs["aux_b"]).reshape(-1)[0])]], np.float32)
    return packed


def kernel(**inputs):
    global _NC, _PACKED
    from concourse.bass_utils import run_bass_kernel_spmd

    if _NC is None:
        _NC = build()
    if _PACKED is None:
        _PACKED = _pack_weights(inputs)
    x_full = np.ascontiguousarray(np.asarray(inputs["x"], np.float32))
    per_core = []
    for c in range(N_CORES):
        m = dict(_PACKED)
        m["x"] = np.ascontiguousarray(x_full[c * B:(c + 1) * B])
        per_core.append(m)
    res = run_bass_kernel_spmd(_NC, per_core, core_ids=list(range(N_CORES)))
    reg = np.concatenate([res.results[c]["out"][0] for c in range(N_CORES)])
    aux = np.concatenate([res.results[c]["out"][1] for c in range(N_CORES)])
    kernel.last_results = res
    return reg, aux
